# revision 34
# baseline (speedup 1.0000x reference)
"""Self-attention kernel for Trainium2 (8 NeuronCores, SPMD).

Problem: X[8192,512], Wq,Wk[512,512]:
    Q = X@Wq ; K = X@Wk ; S = softmax(Q K^T / sqrt(512)) ; out = S @ X

Sharding: rows of Q (query blocks of 1024) across 8 cores; K/V (=X) replicated.

Per-core dataflow (core owns query rows i in [c*1024, (c+1)*1024)):
  Phase P:  M^T = (Wq Wk^T)/sqrt(d)  [512,512]  (16 MMs, f32r, pipelined
            against the 4-chunk weights DMA via 4 concurrent PSUM groups)
            R   = M X_mine^T          [512,1024] (32 MMs, fp16 out)
  B1 (single sweep, both i-halves): for each j-tile (64), for each half:
     S^T tile [j=128, i=512] = sum_e X^T_tile.T R  (4 accumulating fp16
     matmuls; the fp16 LDWEIGHTS hides under the 213ns moving stream;
     both halves share each X^T stationary so xt streams from HBM ONCE)
     -> ACT copy PSUM->SBUF as fp16 with bias -2048 (keeps the top-logit
     fp16 ulp at ~0.25 instead of 2), DVE running max -> mx[h][128,512]
  B2 x2 (emitted back-to-back; h1's chain overlaps B3(h0) on the PE):
     partition-reduce mx via fp16 PE transpose + DVE reduce_max ->
     [1,512]; broadcast via ones outer-product matmul -> b2[128,2,512]
  B3 per half: fp8e4m3 DoubleRow dual-gemm over j double-tiles (256 rows):
     one DVE sub [128,2,512] fp16 (2x mode), one ACT exp -> P~ fp8;
     per i-chunk c: o[c] += P~.T @ X8_dt ; o[c] += P~.T @ Xlo8_dt
     (X = X8 + Xlo8 dual-fp8 so the V-side quantization error cancels)
     + DR N=1 row-sum MMs on the same stationary (p errors cancel o/sum)
  B4: DVE recip(sum[128,4]) -> scaled drain copies split DVE/ACT (fp16),
     per-chunk DMAs to a blocked fp16 output tensor.

All input/output DRAM layouts are host-blocked so every DMA descriptor
moves >=1KB/partition contiguous runs (the single logical DMA queue is
descriptor-FIFO; emission order doubles as a prefetch schedule).

~22 warm-up MMs run against the staging DMAs so the HAM clock gate opens
(K=8/8) before the first real matmul.

Precision: logits see f32r projections + fp16 K^T/R (sigma ~0.26 on a
~120-mean top-2 gap); values see dual-fp8 X (~0.3% residual) and fp8 P~
whose error cancels against the matching fp8 row-sum. rel err ~9e-3.
"""
import sys

sys.path.insert(0, "/opt/trn_rl_repo")

import ml_dtypes
import numpy as np

import concourse.bass as bass
import concourse.mybir as mybir
import concourse.tile as tile
from concourse import bacc
from concourse.bass import ts
from concourse.bass_utils import run_bass_kernel_spmd
from concourse.masks import make_identity

F32 = mybir.dt.float32
F32R = mybir.dt.float32r
F16 = mybir.dt.float16
F8 = mybir.dt.float8e4
AF = mybir.ActivationFunctionType
ALU = mybir.AluOpType
DR = mybir.MatmulPerfMode.DoubleRow

N = 8192
D = 512
NCORES = 8
MY_N = N // NCORES          # 1024 query rows per core
NJT = N // 128              # 64 j-tiles
NDT = N // 256              # 32 j double-tiles (DoubleRow K=256)
NIH = MY_N // 512           # 2 i-halves
NBLK = N // 512             # 16 xt blocks
SHIFT = -2048.0             # logit shift so fp16 st keeps precision near max

_NC_CACHE = None


def _build_nc():
    nc = bacc.Bacc(None, target_bir_lowering=False)

    xt = nc.dram_tensor("xt", [128, NBLK, 4, 512], F16, kind="ExternalInput")  # X^T blocked fp16
    xtmb = nc.dram_tensor("xtmb", [128, 2, 4, 512], F16, kind="ExternalInput")  # X^T slice blocked
    x8 = nc.dram_tensor("x8", [128, NDT, 2, 2, 512], F8, kind="ExternalInput")  # X hi/lo fp8
    wz = nc.dram_tensor("wz", [128, 4, 2, 512], F16, kind="ExternalInput")  # Wq^T/Wk^T by d-chunk
    ob = nc.dram_tensor("ob", [128, 8, 512], F16, kind="ExternalOutput")     # blocked output fp16

    with tile.TileContext(nc) as tc:
        with (
            tc.tile_pool(name="pool", bufs=1) as pool,          # persistent
            tc.tile_pool(name="mtp", bufs=1) as mtp,            # M^T
            tc.tile_pool(name="stream", bufs=3) as stream,      # xt blocks
            tc.tile_pool(name="big", bufs=1) as big,            # xtm + st0 / st1
            tc.tile_pool(name="rpool", bufs=1) as rpool,        # wz then R
            tc.tile_pool(name="xs", bufs=4) as xsp,             # X8 double-tiles (B3)
            tc.tile_pool(name="workd", bufs=4) as workd,        # d
            tc.tile_pool(name="workp", bufs=4) as workp,        # p (fp8 pairs)
            tc.tile_pool(name="osbp", bufs=1) as osbp,
            tc.tile_pool(name="ps_qk", bufs=3, space="PSUM") as ps_qk,
            tc.tile_pool(name="ps_o", bufs=1, space="PSUM") as ps_o,
            tc.tile_pool(name="ps_sum", bufs=1, space="PSUM") as ps_sum,
        ):
            # ---- staging: weights in 4 chunks, blocked xtm, xt prefetch ----
            wz_sb = rpool.tile([128, 4, 2, 512], F16, tag="r")
            for dch in range(4):
                nc.sync.dma_start(wz_sb[:, dch], wz[:, dch])
            mt_sb = mtp.tile([128, 4, 512], F16, tag="mt")
            xtm_sb = big.tile([128, 2, 4, 512], F16, tag="big")
            xt_pf = []  # (blk, tile) staged xt blocks
            nc.sync.dma_start(xtm_sb[:, 0], xtmb[:, 0])
            nc.sync.dma_start(xtm_sb[:, 1], xtmb[:, 1])
            for pfb in range(3):
                t_ = stream.tile([128, 4, 512], F16, tag="stream")
                nc.sync.dma_start(t_[:], xt[:, pfb, :, :])
                xt_pf.append((pfb, t_))

            # ---- Phase P1: M^T = (Wq Wk^T)/sqrt(D), d-chunk outer so the
            #      matmuls pipeline against the weight-chunk DMAs; warm-up
            #      MMs interleaved so the PE never idles on a chunk wait ----
            scale = 1.0 / float(np.sqrt(D))
            mt_ps = []
            for fc in range(4):
                mtb = ps_o.tile([128, 512], F32, tag=f"o{fc}", name=f"mtb{fc}")
                mt_ps.append(mtb)
            for dch in range(4):
                for fc in range(4):
                    nc.tensor.matmul(
                        mt_ps[fc][:],
                        wz_sb[:, dch, 0, ts(fc, 128)],
                        wz_sb[:, dch, 1, :],
                        start=(dch == 0),
                        stop=(dch == 3),
                    )
            for fc in range(4):
                nc.scalar.activation(
                    mt_sb[:, fc, :], mt_ps[fc][:], AF.Copy, bias=0.0, scale=scale
                )

            # ---- Phase P2: R = M X_mine^T  (r_sb reuses wz_sb's memory;
            #      fp16 so B1's moving stream can pair with fp16 weights) ----
            r_sb = rpool.tile([128, 4, MY_N], F16, tag="r")
            for ih in range(NIH):
                for ech in range(4):
                    r_ps = ps_qk.tile([128, 512], F32, tag="qk")
                    for fch in range(4):
                        nc.tensor.matmul(
                            r_ps[:],
                            mt_sb[:, fch, ts(ech, 128)],
                            xtm_sb[:, ih, fch, :],
                            start=(fch == 0),
                            stop=(fch == 3),
                        )
                    nc.scalar.copy(r_sb[:, ech, ts(ih, 512)], r_ps[:])

            # ---- constants (emitted after P so P1's first LDW waits only
            #      the wz DMA, not the DVE init chain; these run during P) ----
            ones_row_f32 = pool.tile([1, 512], F32)
            nc.vector.memset(ones_row_f32[:], 1.0)
            ones_row = pool.tile([1, 128], F32R)   # lhsT for fillers
            nc.vector.tensor_copy(ones_row[:], ones_row_f32[:, 0:128])
            ones_512 = pool.tile([1, 512], F32R)   # filler moving operand
            nc.vector.tensor_copy(ones_512[:], ones_row_f32[:])
            ones_row16 = pool.tile([1, 128], F16)  # lhsT for b broadcast
            nc.vector.tensor_copy(ones_row16[:], ones_row_f32[:, 0:128])
            ones_f32 = pool.tile([128, 2], F32)
            nc.vector.memset(ones_f32[:], 1.0)
            ones8 = pool.tile([128, 2, 1], F8)     # DR rhs for row sums
            nc.vector.tensor_copy(ones8[:, :, 0], ones_f32[:])
            ident = pool.tile([128, 128], F32)
            make_identity(nc, ident[:])
            ident16 = pool.tile([128, 128], F16)
            nc.vector.tensor_copy(ident16[:], ident[:])

            def warmup(n, nn=512):
                # PE filler MMs: bridge serial-chain windows so the HAM MID
                # activity monitor (~1.7us at 2.4GHz) never sees the PE idle
                # long enough to re-throttle the clock to 1.2GHz
                for wu in range(n):
                    wu_ps = ps_qk.tile([128, 512], F32, tag="qk")
                    nc.tensor.matmul(
                        wu_ps[:, 0:nn], ones_row[:], ones_512[:, 0:nn],
                        start=True, stop=True,
                    )

            # ---- helpers ----
            def finalize_max(mx, tag):
                """mx[128,512] fp16 -> b2[128,2,512] fp16 broadcast of max.
                Per-chunk mcol tiles keep the 4 transpose->reduce pairs free
                of write-write serialization so they pipeline."""
                mcols = []
                for c in range(4):
                    tp_ps = ps_qk.tile([128, 128], F16, tag="qk")
                    nc.tensor.transpose(tp_ps[:], mx[:, ts(c, 128)], ident16[:])
                    mcol = pool.tile([128, 1], F16, tag=f"mcol{tag}{c}")
                    nc.vector.reduce_max(
                        mcol[:], tp_ps[:], axis=mybir.AxisListType.X
                    )
                    mcols.append(mcol)
                mrow_ps = ps_qk.tile([1, 512], F16, tag="qk")
                for c in range(4):
                    nc.tensor.transpose(
                        mrow_ps[:, ts(c, 128)], mcols[c][:], ident16[:]
                    )
                mrow = pool.tile([1, 512], F16, tag=f"mrow{tag}")
                nc.scalar.copy(mrow[:], mrow_ps[:])
                b_ps = ps_qk.tile([128, 512], F32, tag="qk")
                nc.tensor.matmul(
                    b_ps[:], ones_row16[:], mrow[:], start=True, stop=True
                )
                b2 = pool.tile([128, 2, 512], F16, tag=f"b2{tag}")
                nc.scalar.copy(b2[:, 0, :], b_ps[:])
                nc.scalar.copy(b2[:, 1, :], b_ps[:])
                return b2

            def b1_phase_both(st0, st1, mx0, mx1, pf):
                """Single B1 sweep computing both i-halves per xt block."""
                for blk in range(NBLK):
                    if pf and pf[0][0] == blk:
                        xt_blk = pf.pop(0)[1]
                    else:
                        xt_blk = stream.tile([128, 4, 512], F16, tag="stream")
                        nc.sync.dma_start(xt_blk[:], xt[:, blk, :, :])
                    if blk in (12, 14):
                        # stage the first x8 double-tiles for B3(h0)
                        nd = 0 if blk == 12 else 1
                        t_ = xsp.tile([128, 2, 2, 512], F8, tag="x")
                        nc.sync.dma_start(t_[:], x8[:, nd])
                        x_pf0.append((nd, t_))
                    for t in range(4):
                        jt = blk * 4 + t
                        for st, mx, h in ((st0, mx0, 0), (st1, mx1, 1)):
                            s_ps = ps_qk.tile([128, 512], F32, tag="qk")
                            for e in range(4):
                                nc.tensor.matmul(
                                    s_ps[:],
                                    xt_blk[:, e, ts(t, 128)],
                                    r_sb[:, e, ts(h, 512)],
                                    start=(e == 0),
                                    stop=(e == 3),
                                )
                            nc.scalar.activation(
                                st[:, jt, :], s_ps[:], AF.Copy, bias=SHIFT
                            )
                            if jt == 0:
                                nc.vector.tensor_copy(mx[:], st[:, jt, :])
                            else:
                                nc.vector.tensor_tensor(
                                    mx[:], mx[:], st[:, jt, :], op=ALU.max
                                )

            def arm_b3():
                o_ps = []
                for c in range(4):
                    o_bank = ps_o.tile([128, 512], F32, tag=f"o{c}", name=f"o_bank{c}")
                    o_ps.append(o_bank)
                sum_ps = ps_sum.tile([128, 4], F32, tag="sum")
                return o_ps, sum_ps

            def b3_prep(st, b2, dt, pf):
                """x8 fetch + subtract + exp for one double-tile."""
                if pf and pf[0][0] == dt:
                    x_dt = pf.pop(0)[1]
                else:
                    x_dt = xsp.tile([128, 2, 2, 512], F8, tag="x")
                    nc.sync.dma_start(x_dt[:], x8[:, dt])
                d_t = workd.tile([128, 2, 512], F16, tag="d")
                nc.vector.tensor_tensor(
                    d_t[:], st[:, 2 * dt : 2 * dt + 2, :], b2[:], op=ALU.subtract
                )
                p_t = workp.tile([128, 2, 512], F8, tag="p")
                nc.scalar.activation(p_t[:], d_t[:], AF.Exp)
                return x_dt, p_t

            def b3_phase(h, st, b2, o_ps, sum_ps, pf, pre=None):
                """fp8 DoubleRow dual-gemm over j double-tiles."""
                for dt in range(NDT):
                    if dt == 0 and pre is not None:
                        x_dt, p_t = pre
                    else:
                        x_dt, p_t = b3_prep(st, b2, dt, pf)
                    if h == 0 and dt in (28, 30):
                        # stage B3(h1)'s first x8 tiles late in the h0 stream
                        nd = 0 if dt == 28 else 1
                        t_ = xsp.tile([128, 2, 2, 512], F8, tag="x")
                        nc.sync.dma_start(t_[:], x8[:, nd])
                        x_pf1.append((nd, t_))
                    first = dt == 0
                    last = dt == NDT - 1
                    for c in range(4):
                        lhsT = p_t[:, :, ts(c, 128)]
                        nc.tensor.matmul(
                            o_ps[c][:], lhsT, x_dt[:, 0],
                            start=first, stop=False, perf_mode=DR,
                        )
                        nc.tensor.matmul(
                            sum_ps[:, c : c + 1], lhsT, ones8[:],
                            start=(first and c == 0), stop=(last and c == 3),
                            perf_mode=DR,
                        )
                        nc.tensor.matmul(
                            o_ps[c][:], lhsT, x_dt[:, 1],
                            start=False, stop=last, perf_mode=DR,
                        )

            def b4_drain(h, o_ps, sum_ps):
                """Normalize + evacuate o: copies split DVE/ACT so neither
                engine's FIFO stalls the next phase's subtract/exp chain."""
                rec = pool.tile([128, 4], F32, tag=f"rec{h}")
                nc.vector.reciprocal(rec[:], sum_ps[:])
                o_sb = osbp.tile([128, 4, 512], F16, tag="osb")
                for c in range(4):
                    if c < 2:
                        nc.vector.tensor_scalar_mul(
                            o_sb[:, c, :], o_ps[c][:], rec[:, c : c + 1]
                        )
                    else:
                        nc.scalar.activation(
                            o_sb[:, c, :], o_ps[c][:], AF.Copy,
                            bias=0.0, scale=rec[:, c : c + 1],
                        )
                    nc.sync.dma_start(
                        ob[:, h * 4 + c : h * 4 + c + 1, :], o_sb[:, c : c + 1, :]
                    )

            # ---- main schedule ----
            x_pf0 = []
            x_pf1 = []
            mx0 = pool.tile([128, 512], F16, tag="mx0")
            mx1 = pool.tile([128, 512], F16, tag="mx1")
            st0 = big.tile([128, NJT, 512], F16, tag="big")
            st1 = big.tile([128, NJT, 512], F16, tag="big2")

            b1_phase_both(st0, st1, mx0, mx1, xt_pf)
            # keep-warm MMs threaded through the finalize chains: the serial
            # transpose->reduce->broadcast dependency leaves the PE sparse for
            # ~2.5us, long enough for the HAM MID window to re-throttle the
            # clock to 1.2GHz right as B3(h0) ramps
            b2_0 = finalize_max(mx0, "0")
            b2_1 = finalize_max(mx1, "1")
            warmup(12, nn=256)

            o_ps0, sum_ps0 = arm_b3()
            b3_phase(0, st0, b2_0, o_ps0, sum_ps0, x_pf0)
            # prep h1's first double-tile before draining h0 so the DVE/ACT
            # chain for B3(h1) overlaps the h0 drain copies
            pre1 = b3_prep(st1, b2_1, 0, x_pf1)
            b4_drain(0, o_ps0, sum_ps0)

            o_ps1, sum_ps1 = arm_b3()
            b3_phase(1, st1, b2_1, o_ps1, sum_ps1, x_pf1, pre=pre1)
            b4_drain(1, o_ps1, sum_ps1)

    nc.compile()
    return nc


def _get_nc():
    global _NC_CACHE
    if _NC_CACHE is None:
        _NC_CACHE = _build_nc()
    return _NC_CACHE


def kernel(rotation_params, entangle_params, inputs, _trace=False, _trace_kwargs=None):
    X = np.ascontiguousarray(inputs, dtype=np.float32)
    Wq = np.ascontiguousarray(rotation_params, dtype=np.float32)
    Wk = np.ascontiguousarray(entangle_params, dtype=np.float32)
    XT = np.ascontiguousarray(X.T)
    # blocked layouts: >=1KB runs/partition per DMA descriptor
    XTB = np.ascontiguousarray(
        XT.reshape(4, 128, 16, 512).transpose(1, 2, 0, 3).astype(np.float16)
    )
    f8 = ml_dtypes.float8_e4m3
    X8 = X.astype(f8)
    XLO8 = (X - X8.astype(np.float32)).astype(f8)
    # x8[p, dt, hl, pair, d] = {X8,XLO8}[hl][dt*256 + pair*128 + p, d]
    X8B = np.ascontiguousarray(
        np.stack([X8, XLO8], axis=0)
        .reshape(2, NDT, 2, 128, 512)
        .transpose(3, 1, 0, 2, 4)
    )
    # wz[p, dch, 0/1, f] = W{q,k}^T[dch*128+p, f]
    WZ = np.ascontiguousarray(
        np.stack(
            [
                Wq.T.reshape(4, 128, 512),
                Wk.T.reshape(4, 128, 512),
            ],
            axis=2,
        ).transpose(1, 0, 2, 3).astype(np.float16)
    )

    in_maps = []
    for c in range(NCORES):
        xtm = XT[:, c * MY_N : (c + 1) * MY_N]
        # xtmb[p, ih, fc, i] = xtm[fc*128+p, ih*512+i]
        XTMB = np.ascontiguousarray(
            xtm.reshape(4, 128, 2, 512).transpose(1, 2, 0, 3).astype(np.float16)
        )
        in_maps.append({"xt": XTB, "xtmb": XTMB, "x8": X8B, "wz": WZ})

    nc = _get_nc()
    kw = {}
    if _trace:
        kw["trace"] = True
        kw.update(_trace_kwargs or {})
    br = run_bass_kernel_spmd(nc, in_maps, core_ids=list(range(NCORES)), **kw)
    # ob[p, hc, d] -> out[hc*128+p, d]
    out = np.concatenate(
        [
            np.asarray(r["ob"]).astype(np.float32).transpose(1, 0, 2).reshape(MY_N, D)
            for r in br.results
        ],
        axis=0,
    )
    if _trace:
        return out, br
    return out


# revision 35
# speedup vs baseline: 1.0144x; 1.0144x over previous
"""Self-attention kernel for Trainium2 (8 NeuronCores, SPMD).

Problem: X[8192,512], Wq,Wk[512,512]:
    Q = X@Wq ; K = X@Wk ; S = softmax(Q K^T / sqrt(512)) ; out = S @ X

Sharding: rows of Q (query blocks of 1024) across 8 cores; K/V (=X) replicated.

Per-core dataflow (core owns query rows i in [c*1024, (c+1)*1024)):
  Phase P:  M^T = (Wq Wk^T)/sqrt(d)  [512,512]  (16 MMs, f32r, pipelined
            against the 4-chunk weights DMA via 4 concurrent PSUM groups)
            R   = M X_mine^T          [512,1024] (32 MMs, fp16 out)
  B1 (single sweep, both i-halves): for each j-tile (64), for each half:
     S^T tile [j=128, i=512] = sum_e X^T_tile.T R  (4 accumulating fp16
     matmuls; the fp16 LDWEIGHTS hides under the 213ns moving stream;
     both halves share each X^T stationary so xt streams from HBM ONCE)
     -> ACT copy PSUM->SBUF as fp16 with bias -2048 (keeps the top-logit
     fp16 ulp at ~0.25 instead of 2), DVE running max -> mx[h][128,512]
  B2 x2 (emitted back-to-back; h1's chain overlaps B3(h0) on the PE):
     partition-reduce mx via fp16 PE transpose + DVE reduce_max ->
     [1,512]; broadcast via ones outer-product matmul -> b2[128,2,512]
  B3 per half: fp8e4m3 DoubleRow dual-gemm over j double-tiles (256 rows):
     one DVE sub [128,2,512] fp16 (2x mode), one ACT exp -> P~ fp8;
     per i-chunk c: o[c] += P~.T @ X8_dt ; o[c] += P~.T @ Xlo8_dt
     (X = X8 + Xlo8 dual-fp8 so the V-side quantization error cancels)
     + DR N=1 row-sum MMs on the same stationary (p errors cancel o/sum)
  B4: DVE recip(sum[128,4]) -> scaled drain copies split DVE/ACT (fp16),
     per-chunk DMAs to a blocked fp16 output tensor.

All input/output DRAM layouts are host-blocked so every DMA descriptor
moves >=1KB/partition contiguous runs (the single logical DMA queue is
descriptor-FIFO; emission order doubles as a prefetch schedule).

~22 warm-up MMs run against the staging DMAs so the HAM clock gate opens
(K=8/8) before the first real matmul.

Precision: logits see f32r projections + fp16 K^T/R (sigma ~0.26 on a
~120-mean top-2 gap); values see dual-fp8 X (~0.3% residual) and fp8 P~
whose error cancels against the matching fp8 row-sum. rel err ~9e-3.
"""
import sys

sys.path.insert(0, "/opt/trn_rl_repo")

import ml_dtypes
import numpy as np

import concourse.bass as bass
import concourse.mybir as mybir
import concourse.tile as tile
from concourse import bacc
from concourse.bass import ts
from concourse.bass_utils import run_bass_kernel_spmd
from concourse.masks import make_identity

F32 = mybir.dt.float32
F32R = mybir.dt.float32r
F16 = mybir.dt.float16
F8 = mybir.dt.float8e4
AF = mybir.ActivationFunctionType
ALU = mybir.AluOpType
DR = mybir.MatmulPerfMode.DoubleRow

N = 8192
D = 512
NCORES = 8
MY_N = N // NCORES          # 1024 query rows per core
NJT = N // 128              # 64 j-tiles
NDT = N // 256              # 32 j double-tiles (DoubleRow K=256)
NIH = MY_N // 512           # 2 i-halves
NBLK = N // 512             # 16 xt blocks
SHIFT = -2048.0             # logit shift so fp16 st keeps precision near max

_NC_CACHE = None


def _build_nc():
    nc = bacc.Bacc(None, target_bir_lowering=False)

    xt = nc.dram_tensor("xt", [128, NBLK, 4, 512], F16, kind="ExternalInput")  # X^T blocked fp16
    xtmb = nc.dram_tensor("xtmb", [128, 2, 4, 512], F16, kind="ExternalInput")  # X^T slice blocked
    x8 = nc.dram_tensor("x8", [128, NDT, 2, 2, 512], F8, kind="ExternalInput")  # X hi/lo fp8
    wz = nc.dram_tensor("wz", [128, 4, 2, 512], F16, kind="ExternalInput")  # Wq^T/Wk^T by d-chunk
    ob = nc.dram_tensor("ob", [128, 8, 512], F16, kind="ExternalOutput")     # blocked output fp16

    with tile.TileContext(nc) as tc:
        with (
            tc.tile_pool(name="pool", bufs=1) as pool,          # persistent
            tc.tile_pool(name="mtp", bufs=1) as mtp,            # M^T
            tc.tile_pool(name="stream", bufs=3) as stream,      # xt blocks
            tc.tile_pool(name="big", bufs=1) as big,            # xtm + st0 / st1
            tc.tile_pool(name="rpool", bufs=1) as rpool,        # wz then R
            tc.tile_pool(name="xs", bufs=4) as xsp,             # X8 double-tiles (B3)
            tc.tile_pool(name="workd", bufs=4) as workd,        # d
            tc.tile_pool(name="workp", bufs=4) as workp,        # p (fp8 pairs)
            tc.tile_pool(name="osbp", bufs=1) as osbp,
            tc.tile_pool(name="ps_qk", bufs=3, space="PSUM") as ps_qk,
            tc.tile_pool(name="ps_o", bufs=1, space="PSUM") as ps_o,
            tc.tile_pool(name="ps_sum", bufs=1, space="PSUM") as ps_sum,
        ):
            # ---- staging: weights in 4 chunks, blocked xtm, xt prefetch ----
            wz_sb = rpool.tile([128, 4, 2, 512], F16, tag="r")
            for dch in range(4):
                nc.sync.dma_start(wz_sb[:, dch], wz[:, dch])
            mt_sb = mtp.tile([128, 4, 512], F16, tag="mt")
            xtm_sb = big.tile([128, 2, 4, 512], F16, tag="big")
            xt_pf = []  # (blk, tile) staged xt blocks
            nc.sync.dma_start(xtm_sb[:, 0], xtmb[:, 0])
            nc.sync.dma_start(xtm_sb[:, 1], xtmb[:, 1])
            for pfb in range(3):
                t_ = stream.tile([128, 4, 512], F16, tag="stream")
                nc.sync.dma_start(t_[:], xt[:, pfb, :, :])
                xt_pf.append((pfb, t_))

            # ---- Phase P1: M^T = (Wq Wk^T)/sqrt(D), d-chunk outer so the
            #      matmuls pipeline against the weight-chunk DMAs; warm-up
            #      MMs interleaved so the PE never idles on a chunk wait ----
            scale = 1.0 / float(np.sqrt(D))
            mt_ps = []
            for fc in range(4):
                mtb = ps_o.tile([128, 512], F32, tag=f"o{fc}", name=f"mtb{fc}")
                mt_ps.append(mtb)
            for dch in range(4):
                for fc in range(4):
                    nc.tensor.matmul(
                        mt_ps[fc][:],
                        wz_sb[:, dch, 0, ts(fc, 128)],
                        wz_sb[:, dch, 1, :],
                        start=(dch == 0),
                        stop=(dch == 3),
                    )
            for fc in range(4):
                nc.scalar.activation(
                    mt_sb[:, fc, :], mt_ps[fc][:], AF.Copy, bias=0.0, scale=scale
                )

            # ---- Phase P2: R = M X_mine^T  (r_sb reuses wz_sb's memory;
            #      fp16 so B1's moving stream can pair with fp16 weights) ----
            r_sb = rpool.tile([128, 4, MY_N], F16, tag="r")
            for ih in range(NIH):
                for ech in range(4):
                    r_ps = ps_qk.tile([128, 512], F32, tag="qk")
                    for fch in range(4):
                        nc.tensor.matmul(
                            r_ps[:],
                            mt_sb[:, fch, ts(ech, 128)],
                            xtm_sb[:, ih, fch, :],
                            start=(fch == 0),
                            stop=(fch == 3),
                        )
                    nc.scalar.copy(r_sb[:, ech, ts(ih, 512)], r_ps[:])

            # ---- constants (emitted after P so P1's first LDW waits only
            #      the wz DMA, not the DVE init chain; these run during P) ----
            ones_row_f32 = pool.tile([1, 512], F32)
            nc.vector.memset(ones_row_f32[:], 1.0)
            ones_row = pool.tile([1, 128], F32R)   # lhsT for fillers
            nc.vector.tensor_copy(ones_row[:], ones_row_f32[:, 0:128])
            ones_512 = pool.tile([1, 512], F32R)   # filler moving operand
            nc.vector.tensor_copy(ones_512[:], ones_row_f32[:])
            ones_row16 = pool.tile([1, 128], F16)  # lhsT for b broadcast
            nc.vector.tensor_copy(ones_row16[:], ones_row_f32[:, 0:128])
            ones_f32 = pool.tile([128, 2], F32)
            nc.vector.memset(ones_f32[:], 1.0)
            ones8 = pool.tile([128, 2, 1], F8)     # DR rhs for row sums
            nc.vector.tensor_copy(ones8[:, :, 0], ones_f32[:])
            ident = pool.tile([128, 128], F32)
            make_identity(nc, ident[:])
            ident16 = pool.tile([128, 128], F16)
            nc.vector.tensor_copy(ident16[:], ident[:])

            def warmup(n, nn=512):
                # PE filler MMs: bridge serial-chain windows so the HAM MID
                # activity monitor (~1.7us at 2.4GHz) never sees the PE idle
                # long enough to re-throttle the clock to 1.2GHz
                for wu in range(n):
                    wu_ps = ps_qk.tile([128, 512], F32, tag="qk")
                    nc.tensor.matmul(
                        wu_ps[:, 0:nn], ones_row[:], ones_512[:, 0:nn],
                        start=True, stop=True,
                    )

            # ---- helpers ----
            def finalize_max(mx, tag):
                """mx[128,512] fp16 -> b2[128,2,512] fp16 broadcast of max.
                Per-chunk mcol tiles keep the 4 transpose->reduce pairs free
                of write-write serialization so they pipeline."""
                mcols = []
                for c in range(4):
                    tp_ps = ps_qk.tile([128, 128], F16, tag="qk")
                    nc.tensor.transpose(tp_ps[:], mx[:, ts(c, 128)], ident16[:])
                    mcol = pool.tile([128, 1], F16, tag=f"mcol{tag}{c}")
                    nc.vector.reduce_max(
                        mcol[:], tp_ps[:], axis=mybir.AxisListType.X
                    )
                    mcols.append(mcol)
                mrow_ps = ps_qk.tile([1, 512], F16, tag="qk")
                for c in range(4):
                    nc.tensor.transpose(
                        mrow_ps[:, ts(c, 128)], mcols[c][:], ident16[:]
                    )
                mrow = pool.tile([1, 512], F16, tag=f"mrow{tag}")
                nc.scalar.copy(mrow[:], mrow_ps[:])
                b_ps = ps_qk.tile([128, 512], F32, tag="qk")
                nc.tensor.matmul(
                    b_ps[:], ones_row16[:], mrow[:], start=True, stop=True
                )
                b2 = pool.tile([128, 2, 512], F16, tag=f"b2{tag}")
                nc.scalar.copy(b2[:, 0, :], b_ps[:])
                nc.scalar.copy(b2[:, 1, :], b_ps[:])
                return b2

            def b1_phase_both(st0, st1, mx0, mx1, pf):
                """Single B1 sweep computing both i-halves per xt block."""
                for blk in range(NBLK):
                    if pf and pf[0][0] == blk:
                        xt_blk = pf.pop(0)[1]
                    else:
                        xt_blk = stream.tile([128, 4, 512], F16, tag="stream")
                        nc.sync.dma_start(xt_blk[:], xt[:, blk, :, :])
                    if blk in (12, 14):
                        # stage the first x8 double-tiles for B3(h0)
                        nd = 0 if blk == 12 else 1
                        t_ = xsp.tile([128, 2, 2, 512], F8, tag="x")
                        nc.sync.dma_start(t_[:], x8[:, nd])
                        x_pf0.append((nd, t_))
                    for t in range(4):
                        jt = blk * 4 + t
                        for st, mx, h in ((st0, mx0, 0), (st1, mx1, 1)):
                            s_ps = ps_qk.tile([128, 512], F32, tag="qk")
                            for e in range(4):
                                nc.tensor.matmul(
                                    s_ps[:],
                                    xt_blk[:, e, ts(t, 128)],
                                    r_sb[:, e, ts(h, 512)],
                                    start=(e == 0),
                                    stop=(e == 3),
                                )
                            nc.scalar.activation(
                                st[:, jt, :], s_ps[:], AF.Copy, bias=SHIFT
                            )
                            if jt == 0:
                                nc.vector.tensor_copy(mx[:], st[:, jt, :])
                            else:
                                nc.vector.tensor_tensor(
                                    mx[:], mx[:], st[:, jt, :], op=ALU.max
                                )

            def arm_b3():
                o_ps = []
                for c in range(4):
                    o_bank = ps_o.tile([128, 512], F32, tag=f"o{c}", name=f"o_bank{c}")
                    o_ps.append(o_bank)
                sum_ps = ps_sum.tile([128, 4], F32, tag="sum")
                return o_ps, sum_ps

            def b3_prep(st, b2, dt, pf):
                """x8 fetch + subtract + exp for one double-tile."""
                if pf and pf[0][0] == dt:
                    x_dt = pf.pop(0)[1]
                else:
                    x_dt = xsp.tile([128, 2, 2, 512], F8, tag="x")
                    nc.sync.dma_start(x_dt[:], x8[:, dt])
                d_t = workd.tile([128, 2, 512], F16, tag="d")
                nc.vector.tensor_tensor(
                    d_t[:], st[:, 2 * dt : 2 * dt + 2, :], b2[:], op=ALU.subtract
                )
                p_t = workp.tile([128, 2, 512], F8, tag="p")
                nc.scalar.activation(p_t[:], d_t[:], AF.Exp)
                return x_dt, p_t

            def b3_phase(h, st, b2, o_ps, sum_ps, pf, pre=None):
                """fp8 DoubleRow dual-gemm over j double-tiles."""
                for dt in range(NDT):
                    if dt == 0 and pre is not None:
                        x_dt, p_t = pre
                    else:
                        x_dt, p_t = b3_prep(st, b2, dt, pf)
                    if h == 0 and dt in (28, 30):
                        # stage B3(h1)'s first x8 tiles late in the h0 stream
                        nd = 0 if dt == 28 else 1
                        t_ = xsp.tile([128, 2, 2, 512], F8, tag="x")
                        nc.sync.dma_start(t_[:], x8[:, nd])
                        x_pf1.append((nd, t_))
                    first = dt == 0
                    last = dt == NDT - 1
                    for c in range(4):
                        lhsT = p_t[:, :, ts(c, 128)]
                        nc.tensor.matmul(
                            o_ps[c][:], lhsT, x_dt[:, 0],
                            start=first, stop=False, perf_mode=DR,
                        )
                        nc.tensor.matmul(
                            sum_ps[:, c : c + 1], lhsT, ones8[:],
                            start=(first and c == 0), stop=(last and c == 3),
                            perf_mode=DR,
                        )
                        nc.tensor.matmul(
                            o_ps[c][:], lhsT, x_dt[:, 1],
                            start=False, stop=last, perf_mode=DR,
                        )

            def b4_drain(h, o_ps, sum_ps):
                """Normalize + evacuate o: copies split DVE/ACT so neither
                engine's FIFO stalls the next phase's subtract/exp chain."""
                rec = pool.tile([128, 4], F32, tag=f"rec{h}")
                nc.vector.reciprocal(rec[:], sum_ps[:])
                o_sb = osbp.tile([128, 4, 512], F16, tag="osb")
                for c in range(4):
                    if c < 2:
                        nc.vector.tensor_scalar_mul(
                            o_sb[:, c, :], o_ps[c][:], rec[:, c : c + 1]
                        )
                    else:
                        nc.scalar.activation(
                            o_sb[:, c, :], o_ps[c][:], AF.Copy,
                            bias=0.0, scale=rec[:, c : c + 1],
                        )
                    nc.sync.dma_start(
                        ob[:, h * 4 + c : h * 4 + c + 1, :], o_sb[:, c : c + 1, :]
                    )

            # ---- main schedule ----
            x_pf0 = []
            x_pf1 = []
            mx0 = pool.tile([128, 512], F16, tag="mx0")
            mx1 = pool.tile([128, 512], F16, tag="mx1")
            st0 = big.tile([128, NJT, 512], F16, tag="big")
            st1 = big.tile([128, NJT, 512], F16, tag="big2")

            b1_phase_both(st0, st1, mx0, mx1, xt_pf)
            # keep-warm MMs threaded through the finalize chains: the serial
            # transpose->reduce->broadcast dependency leaves the PE sparse for
            # ~2.5us, long enough for the HAM MID window to re-throttle the
            # clock to 1.2GHz right as B3(h0) ramps
            b2_0 = finalize_max(mx0, "0")
            b2_1 = finalize_max(mx1, "1")
            warmup(2)

            o_ps0, sum_ps0 = arm_b3()
            b3_phase(0, st0, b2_0, o_ps0, sum_ps0, x_pf0)
            # prep h1's first double-tile before draining h0 so the DVE/ACT
            # chain for B3(h1) overlaps the h0 drain copies
            pre1 = b3_prep(st1, b2_1, 0, x_pf1)
            b4_drain(0, o_ps0, sum_ps0)

            o_ps1, sum_ps1 = arm_b3()
            b3_phase(1, st1, b2_1, o_ps1, sum_ps1, x_pf1, pre=pre1)
            b4_drain(1, o_ps1, sum_ps1)

    nc.compile()
    return nc


def _get_nc():
    global _NC_CACHE
    if _NC_CACHE is None:
        _NC_CACHE = _build_nc()
    return _NC_CACHE


def kernel(rotation_params, entangle_params, inputs, _trace=False, _trace_kwargs=None):
    X = np.ascontiguousarray(inputs, dtype=np.float32)
    Wq = np.ascontiguousarray(rotation_params, dtype=np.float32)
    Wk = np.ascontiguousarray(entangle_params, dtype=np.float32)
    XT = np.ascontiguousarray(X.T)
    # blocked layouts: >=1KB runs/partition per DMA descriptor
    XTB = np.ascontiguousarray(
        XT.reshape(4, 128, 16, 512).transpose(1, 2, 0, 3).astype(np.float16)
    )
    f8 = ml_dtypes.float8_e4m3
    X8 = X.astype(f8)
    XLO8 = (X - X8.astype(np.float32)).astype(f8)
    # x8[p, dt, hl, pair, d] = {X8,XLO8}[hl][dt*256 + pair*128 + p, d]
    X8B = np.ascontiguousarray(
        np.stack([X8, XLO8], axis=0)
        .reshape(2, NDT, 2, 128, 512)
        .transpose(3, 1, 0, 2, 4)
    )
    # wz[p, dch, 0/1, f] = W{q,k}^T[dch*128+p, f]
    WZ = np.ascontiguousarray(
        np.stack(
            [
                Wq.T.reshape(4, 128, 512),
                Wk.T.reshape(4, 128, 512),
            ],
            axis=2,
        ).transpose(1, 0, 2, 3).astype(np.float16)
    )

    in_maps = []
    for c in range(NCORES):
        xtm = XT[:, c * MY_N : (c + 1) * MY_N]
        # xtmb[p, ih, fc, i] = xtm[fc*128+p, ih*512+i]
        XTMB = np.ascontiguousarray(
            xtm.reshape(4, 128, 2, 512).transpose(1, 2, 0, 3).astype(np.float16)
        )
        in_maps.append({"xt": XTB, "xtmb": XTMB, "x8": X8B, "wz": WZ})

    nc = _get_nc()
    kw = {}
    if _trace:
        kw["trace"] = True
        kw.update(_trace_kwargs or {})
    br = run_bass_kernel_spmd(nc, in_maps, core_ids=list(range(NCORES)), **kw)
    # ob[p, hc, d] -> out[hc*128+p, d]
    out = np.concatenate(
        [
            np.asarray(r["ob"]).astype(np.float32).transpose(1, 0, 2).reshape(MY_N, D)
            for r in br.results
        ],
        axis=0,
    )
    if _trace:
        return out, br
    return out


# revision 36
# speedup vs baseline: 1.0161x; 1.0017x over previous
"""Self-attention kernel for Trainium2 (8 NeuronCores, SPMD).

Problem: X[8192,512], Wq,Wk[512,512]:
    Q = X@Wq ; K = X@Wk ; S = softmax(Q K^T / sqrt(512)) ; out = S @ X

Sharding: rows of Q (query blocks of 1024) across 8 cores; K/V (=X) replicated.

Per-core dataflow (core owns query rows i in [c*1024, (c+1)*1024)):
  Phase P:  M^T = (Wq Wk^T)/sqrt(d)  [512,512]  (16 MMs, f32r, pipelined
            against the 4-chunk weights DMA via 4 concurrent PSUM groups)
            R   = M X_mine^T          [512,1024] (32 MMs, fp16 out)
  B1 (single sweep, both i-halves): for each j-tile (64), for each half:
     S^T tile [j=128, i=512] = sum_e X^T_tile.T R  (4 accumulating fp16
     matmuls; the fp16 LDWEIGHTS hides under the 213ns moving stream;
     both halves share each X^T stationary so xt streams from HBM ONCE)
     -> ACT copy PSUM->SBUF as fp16 with bias -2048 (keeps the top-logit
     fp16 ulp at ~0.25 instead of 2), DVE running max -> mx[h][128,512]
  B2 x2 (emitted back-to-back; h1's chain overlaps B3(h0) on the PE):
     partition-reduce mx via fp16 PE transpose + DVE reduce_max ->
     [1,512]; broadcast via ones outer-product matmul -> b2[128,2,512]
  B3 per half: fp8e4m3 DoubleRow dual-gemm over j double-tiles (256 rows):
     one DVE sub [128,2,512] fp16 (2x mode), one ACT exp -> P~ fp8;
     per i-chunk c: o[c] += P~.T @ X8_dt ; o[c] += P~.T @ Xlo8_dt
     (X = X8 + Xlo8 dual-fp8 so the V-side quantization error cancels)
     + DR N=1 row-sum MMs on the same stationary (p errors cancel o/sum)
  B4: DVE recip(sum[128,4]) -> scaled drain copies split DVE/ACT (fp16),
     per-chunk DMAs to a blocked fp16 output tensor.

All input/output DRAM layouts are host-blocked so every DMA descriptor
moves >=1KB/partition contiguous runs (the single logical DMA queue is
descriptor-FIFO; emission order doubles as a prefetch schedule).

A few PE filler MMs bridge the finalize window so the HAM activity
monitor doesn't re-throttle the PE clock right as B3(h0) ramps.

Precision: logits see fp16 projections + fp16 K^T/R (sigma ~0.3 on a
~120-mean top-2 gap); values see dual-fp8 X (~0.3% residual) and fp8 P~
whose error cancels against the matching fp8 row-sum. rel err ~1.3e-2.
"""
import sys

sys.path.insert(0, "/opt/trn_rl_repo")

import ml_dtypes
import numpy as np

import concourse.bass as bass
import concourse.mybir as mybir
import concourse.tile as tile
from concourse import bacc
from concourse.bass import ts
from concourse.bass_utils import run_bass_kernel_spmd
from concourse.masks import make_identity

F32 = mybir.dt.float32
F32R = mybir.dt.float32r
F16 = mybir.dt.float16
F8 = mybir.dt.float8e4
AF = mybir.ActivationFunctionType
ALU = mybir.AluOpType
DR = mybir.MatmulPerfMode.DoubleRow

N = 8192
D = 512
NCORES = 8
MY_N = N // NCORES          # 1024 query rows per core
NJT = N // 128              # 64 j-tiles
NDT = N // 256              # 32 j double-tiles (DoubleRow K=256)
NIH = MY_N // 512           # 2 i-halves
NBLK = N // 512             # 16 xt blocks
SHIFT = -2048.0             # logit shift so fp16 st keeps precision near max

_NC_CACHE = None


def _build_nc():
    nc = bacc.Bacc(None, target_bir_lowering=False)

    xt = nc.dram_tensor("xt", [128, NBLK, 4, 512], F16, kind="ExternalInput")  # X^T blocked fp16
    xtmb = nc.dram_tensor("xtmb", [128, 2, 4, 512], F16, kind="ExternalInput")  # X^T slice blocked
    x8 = nc.dram_tensor("x8", [128, NDT, 2, 2, 512], F8, kind="ExternalInput")  # X hi/lo fp8
    wz = nc.dram_tensor("wz", [128, 4, 2, 512], F16, kind="ExternalInput")  # Wq^T/Wk^T by d-chunk
    ob = nc.dram_tensor("ob", [128, 8, 512], F16, kind="ExternalOutput")     # blocked output fp16

    with tile.TileContext(nc) as tc:
        with (
            tc.tile_pool(name="pool", bufs=1) as pool,          # persistent
            tc.tile_pool(name="mtp", bufs=1) as mtp,            # M^T
            tc.tile_pool(name="stream", bufs=3) as stream,      # xt blocks
            tc.tile_pool(name="big", bufs=1) as big,            # xtm + st0 / st1
            tc.tile_pool(name="rpool", bufs=1) as rpool,        # wz then R
            tc.tile_pool(name="xs", bufs=4) as xsp,             # X8 double-tiles (B3)
            tc.tile_pool(name="workd", bufs=4) as workd,        # d
            tc.tile_pool(name="workp", bufs=4) as workp,        # p (fp8 pairs)
            tc.tile_pool(name="osbp", bufs=1) as osbp,
            tc.tile_pool(name="ps_qk", bufs=3, space="PSUM") as ps_qk,
            tc.tile_pool(name="ps_o", bufs=1, space="PSUM") as ps_o,
            tc.tile_pool(name="ps_sum", bufs=1, space="PSUM") as ps_sum,
        ):
            # ---- staging: weights in 4 chunks, blocked xtm, xt prefetch ----
            wz_sb = rpool.tile([128, 4, 2, 512], F16, tag="r")
            for dch in range(4):
                nc.sync.dma_start(wz_sb[:, dch], wz[:, dch])
            mt_sb = mtp.tile([128, 4, 512], F16, tag="mt")
            xtm_sb = big.tile([128, 2, 4, 512], F16, tag="big")
            xt_pf = []  # (blk, tile) staged xt blocks
            nc.sync.dma_start(xtm_sb[:, 0], xtmb[:, 0])
            nc.sync.dma_start(xtm_sb[:, 1], xtmb[:, 1])
            for pfb in range(3):
                t_ = stream.tile([128, 4, 512], F16, tag="stream")
                nc.sync.dma_start(t_[:], xt[:, pfb, :, :])
                xt_pf.append((pfb, t_))

            # ---- Phase P1: M^T = (Wq Wk^T)/sqrt(D), d-chunk outer so the
            #      matmuls pipeline against the weight-chunk DMAs; warm-up
            #      MMs interleaved so the PE never idles on a chunk wait ----
            scale = 1.0 / float(np.sqrt(D))
            mt_ps = []
            for fc in range(4):
                mtb = ps_o.tile([128, 512], F32, tag=f"o{fc}", name=f"mtb{fc}")
                mt_ps.append(mtb)
            for dch in range(4):
                for fc in range(4):
                    nc.tensor.matmul(
                        mt_ps[fc][:],
                        wz_sb[:, dch, 0, ts(fc, 128)],
                        wz_sb[:, dch, 1, :],
                        start=(dch == 0),
                        stop=(dch == 3),
                    )
            for fc in range(4):
                nc.scalar.activation(
                    mt_sb[:, fc, :], mt_ps[fc][:], AF.Copy, bias=0.0, scale=scale
                )

            # ---- Phase P2: R = M X_mine^T  (r_sb reuses wz_sb's memory;
            #      fp16 so B1's moving stream can pair with fp16 weights) ----
            r_sb = rpool.tile([128, 4, MY_N], F16, tag="r")
            for ih in range(NIH):
                for ech in range(4):
                    r_ps = ps_qk.tile([128, 512], F32, tag="qk")
                    for fch in range(4):
                        nc.tensor.matmul(
                            r_ps[:],
                            mt_sb[:, fch, ts(ech, 128)],
                            xtm_sb[:, ih, fch, :],
                            start=(fch == 0),
                            stop=(fch == 3),
                        )
                    nc.scalar.copy(r_sb[:, ech, ts(ih, 512)], r_ps[:])

            # ---- constants (emitted after P so P1's first LDW waits only
            #      the wz DMA, not the DVE init chain; these run during P) ----
            ones_row_f32 = pool.tile([1, 512], F32)
            nc.vector.memset(ones_row_f32[:], 1.0)
            ones_row = pool.tile([1, 128], F32R)   # lhsT for fillers
            nc.vector.tensor_copy(ones_row[:], ones_row_f32[:, 0:128])
            ones_512 = pool.tile([1, 512], F32R)   # filler moving operand
            nc.vector.tensor_copy(ones_512[:], ones_row_f32[:])
            ones_row16 = pool.tile([1, 128], F16)  # lhsT for b broadcast
            nc.vector.tensor_copy(ones_row16[:], ones_row_f32[:, 0:128])
            ones_f32 = pool.tile([128, 2], F32)
            nc.vector.memset(ones_f32[:], 1.0)
            ones8 = pool.tile([128, 2, 1], F8)     # DR rhs for row sums
            nc.vector.tensor_copy(ones8[:, :, 0], ones_f32[:])
            ident = pool.tile([128, 128], F32)
            make_identity(nc, ident[:])
            ident16 = pool.tile([128, 128], F16)
            nc.vector.tensor_copy(ident16[:], ident[:])

            def warmup(n, nn=512):
                # PE filler MMs: bridge serial-chain windows so the HAM MID
                # activity monitor (~1.7us at 2.4GHz) never sees the PE idle
                # long enough to re-throttle the clock to 1.2GHz
                for wu in range(n):
                    wu_ps = ps_qk.tile([128, 512], F32, tag="qk")
                    nc.tensor.matmul(
                        wu_ps[:, 0:nn], ones_row[:], ones_512[:, 0:nn],
                        start=True, stop=True,
                    )

            # ---- helpers ----
            def finalize_max(mx, tag):
                """mx[128,512] fp16 -> b2[128,2,512] fp16 broadcast of max.
                Per-chunk mcol tiles keep the 4 transpose->reduce pairs free
                of write-write serialization so they pipeline."""
                mcols = []
                for c in range(4):
                    tp_ps = ps_qk.tile([128, 128], F16, tag="qk")
                    nc.tensor.transpose(tp_ps[:], mx[:, ts(c, 128)], ident16[:])
                    mcol = pool.tile([128, 1], F16, tag=f"mcol{tag}{c}")
                    nc.vector.reduce_max(
                        mcol[:], tp_ps[:], axis=mybir.AxisListType.X
                    )
                    mcols.append(mcol)
                mrow_ps = ps_qk.tile([1, 512], F16, tag="qk")
                for c in range(4):
                    nc.tensor.transpose(
                        mrow_ps[:, ts(c, 128)], mcols[c][:], ident16[:]
                    )
                mrow = pool.tile([1, 512], F16, tag=f"mrow{tag}")
                nc.scalar.copy(mrow[:], mrow_ps[:])
                b_ps = ps_qk.tile([128, 512], F32, tag="qk")
                nc.tensor.matmul(
                    b_ps[:], ones_row16[:], mrow[:], start=True, stop=True
                )
                b2 = pool.tile([128, 2, 512], F16, tag=f"b2{tag}")
                nc.scalar.copy(b2[:, 0, :], b_ps[:])
                nc.scalar.copy(b2[:, 1, :], b_ps[:])
                return b2

            def b1_phase_both(st0, st1, mx0, mx1, pf):
                """Single B1 sweep computing both i-halves per xt block."""
                for blk in range(NBLK):
                    if pf and pf[0][0] == blk:
                        xt_blk = pf.pop(0)[1]
                    else:
                        xt_blk = stream.tile([128, 4, 512], F16, tag="stream")
                        nc.sync.dma_start(xt_blk[:], xt[:, blk, :, :])
                    if blk in (12, 14):
                        # stage the first x8 double-tiles for B3(h0)
                        nd = 0 if blk == 12 else 1
                        t_ = xsp.tile([128, 2, 2, 512], F8, tag="x")
                        nc.sync.dma_start(t_[:], x8[:, nd])
                        x_pf0.append((nd, t_))
                    for t in range(4):
                        jt = blk * 4 + t
                        for st, mx, h in ((st0, mx0, 0), (st1, mx1, 1)):
                            s_ps = ps_qk.tile([128, 512], F32, tag="qk")
                            for e in range(4):
                                nc.tensor.matmul(
                                    s_ps[:],
                                    xt_blk[:, e, ts(t, 128)],
                                    r_sb[:, e, ts(h, 512)],
                                    start=(e == 0),
                                    stop=(e == 3),
                                )
                            nc.scalar.activation(
                                st[:, jt, :], s_ps[:], AF.Copy, bias=SHIFT
                            )
                            if jt == 0:
                                nc.vector.tensor_copy(mx[:], st[:, jt, :])
                            else:
                                nc.vector.tensor_tensor(
                                    mx[:], mx[:], st[:, jt, :], op=ALU.max
                                )

            def arm_b3():
                o_ps = []
                for c in range(4):
                    o_bank = ps_o.tile([128, 512], F32, tag=f"o{c}", name=f"o_bank{c}")
                    o_ps.append(o_bank)
                sum_ps = ps_sum.tile([128, 4], F32, tag="sum")
                return o_ps, sum_ps

            def b3_prep(st, b2, dt, pf):
                """x8 fetch + subtract + exp for one double-tile."""
                if pf and pf[0][0] == dt:
                    x_dt = pf.pop(0)[1]
                else:
                    x_dt = xsp.tile([128, 2, 2, 512], F8, tag="x")
                    nc.sync.dma_start(x_dt[:], x8[:, dt])
                d_t = workd.tile([128, 2, 512], F16, tag="d")
                nc.vector.tensor_tensor(
                    d_t[:], st[:, 2 * dt : 2 * dt + 2, :], b2[:], op=ALU.subtract
                )
                p_t = workp.tile([128, 2, 512], F8, tag="p")
                nc.scalar.activation(p_t[:], d_t[:], AF.Exp)
                return x_dt, p_t

            def b3_phase(h, st, b2, o_ps, sum_ps, pf, pre=None):
                """fp8 DoubleRow dual-gemm over j double-tiles."""
                for dt in range(NDT):
                    if dt == 0 and pre is not None:
                        x_dt, p_t = pre
                    else:
                        x_dt, p_t = b3_prep(st, b2, dt, pf)
                    if h == 0 and dt in (28, 30):
                        # stage B3(h1)'s first x8 tiles late in the h0 stream
                        nd = 0 if dt == 28 else 1
                        t_ = xsp.tile([128, 2, 2, 512], F8, tag="x")
                        nc.sync.dma_start(t_[:], x8[:, nd])
                        x_pf1.append((nd, t_))
                    first = dt == 0
                    last = dt == NDT - 1
                    for c in range(4):
                        lhsT = p_t[:, :, ts(c, 128)]
                        nc.tensor.matmul(
                            o_ps[c][:], lhsT, x_dt[:, 0],
                            start=first, stop=False, perf_mode=DR,
                        )
                        nc.tensor.matmul(
                            sum_ps[:, c : c + 1], lhsT, ones8[:],
                            start=(first and c == 0), stop=(last and c == 3),
                            perf_mode=DR,
                        )
                        nc.tensor.matmul(
                            o_ps[c][:], lhsT, x_dt[:, 1],
                            start=False, stop=last, perf_mode=DR,
                        )

            def b4_drain(h, o_ps, sum_ps):
                """Normalize + evacuate o: copies split DVE/ACT so neither
                engine's FIFO stalls the next phase's subtract/exp chain."""
                rec = pool.tile([128, 4], F32, tag=f"rec{h}")
                nc.vector.reciprocal(rec[:], sum_ps[:])
                o_sb = osbp.tile([128, 4, 512], F16, tag="osb")
                for c in range(4):
                    if c < 2:
                        nc.vector.tensor_scalar_mul(
                            o_sb[:, c, :], o_ps[c][:], rec[:, c : c + 1]
                        )
                    else:
                        nc.scalar.activation(
                            o_sb[:, c, :], o_ps[c][:], AF.Copy,
                            bias=0.0, scale=rec[:, c : c + 1],
                        )
                    nc.sync.dma_start(
                        ob[:, h * 4 + c : h * 4 + c + 1, :], o_sb[:, c : c + 1, :]
                    )

            # ---- main schedule ----
            x_pf0 = []
            x_pf1 = []
            mx0 = pool.tile([128, 512], F16, tag="mx0")
            mx1 = pool.tile([128, 512], F16, tag="mx1")
            st0 = big.tile([128, NJT, 512], F16, tag="big")
            st1 = big.tile([128, NJT, 512], F16, tag="big2")

            b1_phase_both(st0, st1, mx0, mx1, xt_pf)
            # keep-warm MMs threaded through the finalize chains: the serial
            # transpose->reduce->broadcast dependency leaves the PE sparse for
            # ~2.5us, long enough for the HAM MID window to re-throttle the
            # clock to 1.2GHz right as B3(h0) ramps
            b2_0 = finalize_max(mx0, "0")
            b2_1 = finalize_max(mx1, "1")
            warmup(2)

            o_ps0, sum_ps0 = arm_b3()
            b3_phase(0, st0, b2_0, o_ps0, sum_ps0, x_pf0)
            # prep h1's first double-tile before draining h0 so the DVE/ACT
            # chain for B3(h1) overlaps the h0 drain copies
            pre1 = b3_prep(st1, b2_1, 0, x_pf1)
            b4_drain(0, o_ps0, sum_ps0)

            o_ps1, sum_ps1 = arm_b3()
            b3_phase(1, st1, b2_1, o_ps1, sum_ps1, x_pf1, pre=pre1)
            b4_drain(1, o_ps1, sum_ps1)

    nc.compile()
    return nc


def _get_nc():
    global _NC_CACHE
    if _NC_CACHE is None:
        _NC_CACHE = _build_nc()
    return _NC_CACHE


def kernel(rotation_params, entangle_params, inputs, _trace=False, _trace_kwargs=None):
    X = np.ascontiguousarray(inputs, dtype=np.float32)
    Wq = np.ascontiguousarray(rotation_params, dtype=np.float32)
    Wk = np.ascontiguousarray(entangle_params, dtype=np.float32)
    XT = np.ascontiguousarray(X.T)
    # blocked layouts: >=1KB runs/partition per DMA descriptor
    XTB = np.ascontiguousarray(
        XT.reshape(4, 128, 16, 512).transpose(1, 2, 0, 3).astype(np.float16)
    )
    f8 = ml_dtypes.float8_e4m3
    X8 = X.astype(f8)
    XLO8 = (X - X8.astype(np.float32)).astype(f8)
    # x8[p, dt, hl, pair, d] = {X8,XLO8}[hl][dt*256 + pair*128 + p, d]
    X8B = np.ascontiguousarray(
        np.stack([X8, XLO8], axis=0)
        .reshape(2, NDT, 2, 128, 512)
        .transpose(3, 1, 0, 2, 4)
    )
    # wz[p, dch, 0/1, f] = W{q,k}^T[dch*128+p, f]
    WZ = np.ascontiguousarray(
        np.stack(
            [
                Wq.T.reshape(4, 128, 512),
                Wk.T.reshape(4, 128, 512),
            ],
            axis=2,
        ).transpose(1, 0, 2, 3).astype(np.float16)
    )

    in_maps = []
    for c in range(NCORES):
        xtm = XT[:, c * MY_N : (c + 1) * MY_N]
        # xtmb[p, ih, fc, i] = xtm[fc*128+p, ih*512+i]
        XTMB = np.ascontiguousarray(
            xtm.reshape(4, 128, 2, 512).transpose(1, 2, 0, 3).astype(np.float16)
        )
        in_maps.append({"xt": XTB, "xtmb": XTMB, "x8": X8B, "wz": WZ})

    nc = _get_nc()
    kw = {}
    if _trace:
        kw["trace"] = True
        kw.update(_trace_kwargs or {})
    br = run_bass_kernel_spmd(nc, in_maps, core_ids=list(range(NCORES)), **kw)
    # ob[p, hc, d] -> out[hc*128+p, d]
    out = np.concatenate(
        [
            np.asarray(r["ob"]).astype(np.float32).transpose(1, 0, 2).reshape(MY_N, D)
            for r in br.results
        ],
        axis=0,
    )
    if _trace:
        return out, br
    return out
